# revision 40
# baseline (speedup 1.0000x reference)
"""nn_Attention Trainium2 Bass kernel — data-parallel over batch on 8 NeuronCores.

Per core (one batch element): full attention
  qh = q@Wq + bq; kh = k@Wk + bk; vh = kh@Wv + bv
  scores = qh@kh.T  (+ mask -> -10000); probs = softmax(scores)
  out = (probs @ vh) @ Wo + bo

Weight-folding formulation (exact up to softmax shift invariance):
  softmax rows are invariant to per-row constants, so
    scores ~ q @ A @ k.T + c_row,   A = Wq @ Wk.T,  c_k = (Wk @ bq) . k_col
  (the q@Wq@bk and bq@bk terms are per-row constants and cancel), and since
  probs rows sum to 1,
    out = probs @ (k @ C) + bo2,    C = Wk@Wv@Wo,  bo2 = bk@Wv@Wo + bv@Wo + bo.
  A, C, bo2 and the c row are folded on the host (weights-only GEMMs + one
  matvec); the device never computes qh, kh, vh or the final Wo projection.
  This removes ~2/5 of the PE work vs the unfolded kernel.

Other optimizations:
  * k-compaction: masked-out k columns contribute nothing; the host gathers
    only unmasked columns (~1058 of 2048) and pads to lkc (mult of 128).
    Padding columns have ktc=0 -> z=0, vh'=0 and bias -10000 -> prob 0.
  * Everything streams through the PE in fp16 (1 cycle/row at any width);
    z and vh' intermediates are SBUF-resident fp16, psum accum is f32.
  * The per-k bias row is host-computed, replicated to 128 partitions, and
    DMA'd; it is added into the scores psum in place by the DVE.
  * AV directly produces the transposed output (v on partitions), so each
    AV psum group is activated (+bo2) and DMA'd out immediately.

Device-side per core:
  phase A per 512-col k-slice:  z[i,kc]   = at-tiles.T @ ktb   [fp16]
                                vh'[kc,v] = ktb-tiles.T @ ct   [fp16]
  phase B per 1024-col q-block, per q-tile:
     scores psum = qtb-tiles.T @ z-slices ; += biasrep (DVE, in place)
     softmax rowwise (max, exp with accum-sum, reciprocal, scale)
     probsT via PE transposes (fp16)
     out[v,q] psum = vh'-tiles.T @ probsT ; activation +bo2 -> fp16 -> DMA
Host: out[b] = outT.T (cast f32)
"""
import numpy as np

import concourse.bass as bass
import concourse.mybir as mybir
from concourse import bacc, tile
from concourse.bass_utils import run_bass_kernel_spmd
from concourse.masks import make_identity

B, L, D, H = 8, 2048, 1024, 1024
P = 128
F32 = mybir.dt.float32
FP16 = mybir.dt.float16
AF = mybir.ActivationFunctionType
AX = mybir.AxisListType

QBLK = 1024         # q columns per main-loop block
NQT = QBLK // P     # 8 q tiles per block
NQB = L // QBLK     # 2
DT = D // P         # 8 d tiles

LKC_DEFAULT = 1152  # padded compact-k length (multiple of 128)


def build_nc(lkc=LKC_DEFAULT):
    assert lkc % P == 0 and 256 <= lkc <= 2048
    # k-slices: 512-wide chunks (last one lkc-1024 or remainder)
    kslices = []
    off = 0
    while off < lkc:
        w = min(512, lkc - off)
        kslices.append((off, w))
        off += w
    nkt = lkc // P
    sps_bufs = 2 if lkc <= 1408 else 1  # 2*lkc*4 + ~5KB must fit 16KB PSUM

    nc = bacc.Bacc("TRN2", target_bir_lowering=False, debug=False, num_devices=8)
    qt_d = nc.dram_tensor("qt", [D, L], FP16, kind="ExternalInput").ap()
    ktc_d = nc.dram_tensor("ktc", [D, lkc], FP16, kind="ExternalInput").ap()
    at_d = nc.dram_tensor("at", [D, D], FP16, kind="ExternalInput").ap()
    ct_d = nc.dram_tensor("ct", [D, D], FP16, kind="ExternalInput").ap()
    bo2_d = nc.dram_tensor("bo2", [P, DT], F32, kind="ExternalInput").ap()
    brep_d = nc.dram_tensor("brep", [P, lkc], FP16, kind="ExternalInput").ap()
    out_d = nc.dram_tensor("out", [D, L], FP16, kind="ExternalOutput").ap()

    with tile.TileContext(nc) as tc:
        with tc.tile_pool(name="const", bufs=1) as cp, \
             tc.tile_pool(name="persist", bufs=1) as pp_:
            bo2_t = cp.tile([P, DT], F32)
            ident_f = cp.tile([P, P], F32)
            make_identity(nc, ident_f)
            identh_t = cp.tile([P, P], FP16)
            nc.vector.tensor_copy(identh_t, ident_f)
            biasrep_t = cp.tile([P, lkc], FP16)


            # persistent across the whole kernel.  z is split so every moving
            # slice starts 2KB-aligned in SBUF (zbB is padded to 1024 cols so
            # its per-i row stride stays 2048B).
            zbA = pp_.tile([P, DT, 1024], FP16)               # z k-cols [0,1024)
            zbB = None
            if lkc > 1024:
                zbB = pp_.tile([P, DT, 1024], FP16)           # z k-cols [1024,lkc)
            vh_t = pp_.tile([P, nkt, H], FP16)                # vh', k on partitions

            def z_slice(i, koff, kw):
                if koff < 1024:
                    assert koff + kw <= 1024
                    return zbA[:, i, koff:koff + kw]
                return zbB[:, i, koff - 1024:koff - 1024 + kw]

            # Pre-reserve right-side SBUF for the first q-block and load it
            # at t~0 on an otherwise-idle queue.
            bq1_cm = tc.tile_pool(name="bq1", bufs=1, side="right")
            qp1 = bq1_cm.__enter__()
            first_q = qp1.tile([P, DT, QBLK], FP16, tag="qtb0")

            # ---------- A: z for all k-slices, then vh' (ktb slices retained)
            # DMA priority: the first ~10us of HBM bandwidth belongs to at +
            # ktb slice 0 exclusively (they gate the first z matmuls); ct
            # queues behind ktb on gpsimd, first_q behind at on sync.
            with tc.tile_pool(name="aw", bufs=1) as awp, \
                 tc.tile_pool(name="akt", bufs=1) as aktp, \
                 tc.tile_pool(name="aps", bufs=3, space="PSUM") as apsp:
                # at and ktb live in per-d tiles so each z matmul depends on
                # exactly one (at[j], ktb[j]) DMA pair, not the whole batch
                at_ts = [awp.tile([P, D], FP16, tag=f"at{d}", name=f"at{d}")
                         for d in range(DT)]
                ct_t = awp.tile([P, DT, D], FP16, tag="ct")
                for d in range(DT):
                    eng = nc.sync if d % 2 == 0 else nc.scalar
                    eng.dma_start(out=at_ts[d], in_=at_d[d * P:(d + 1) * P, :])
                ktbs = []
                for lb, (koff, kw) in enumerate(kslices):
                    row = []
                    for d in range(DT):
                        kt_ = aktp.tile([P, 512], FP16, tag=f"ktb{lb}_{d}",
                                        name=f"ktb{lb}_{d}")
                        nc.gpsimd.dma_start(out=kt_[:, 0:kw],
                                            in_=ktc_d[d * P:(d + 1) * P, koff:koff + kw])
                        row.append(kt_)
                    ktbs.append(row)
                for d in range(DT):
                    nc.sync.dma_start(out=first_q[:, d],
                                      in_=qt_d[d * P:(d + 1) * P, 0:QBLK])
                for i in range(DT):
                    nc.gpsimd.dma_start(out=ct_t[:, i], in_=ct_d[i * P:(i + 1) * P, :])
                nc.scalar.dma_start(out=bo2_t, in_=bo2_d)
                nc.scalar.dma_start(out=biasrep_t, in_=brep_d)
                # z
                for lb, (koff, kw) in enumerate(kslices):
                    for i in range(DT):
                        ps = apsp.tile([P, 512], F32, tag="ps")
                        for j in range(DT):
                            nc.tensor.matmul(ps[:, 0:kw],
                                             at_ts[j][:, i * P:(i + 1) * P],
                                             ktbs[lb][j][:, 0:kw],
                                             start=(j == 0), stop=(j == DT - 1))
                        nc.scalar.activation(z_slice(i, koff, kw), ps[:, 0:kw],
                                             AF.Copy)
                # vh'
                for lb, (koff, kw) in enumerate(kslices):
                    for lt in range(kw // P):
                        kt_idx = (koff // P) + lt
                        for vb in range(2):
                            ps = apsp.tile([P, 512], F32, tag="ps")
                            for d in range(DT):
                                nc.tensor.matmul(
                                    ps, ktbs[lb][d][:, lt * P:(lt + 1) * P],
                                    ct_t[:, d, vb * 512:(vb + 1) * 512],
                                    start=(d == 0), stop=(d == DT - 1))
                            nc.scalar.activation(
                                vh_t[:, kt_idx, vb * 512:(vb + 1) * 512], ps, AF.Copy)

            # ---------- B: attention per q-block
            with tc.tile_pool(name="bq2", bufs=1) as qp, \
                 tc.tile_pool(name="bsm", bufs=3) as smp, \
                 tc.tile_pool(name="bpt", bufs=2) as ptp, \
                 tc.tile_pool(name="bst", bufs=4) as stp, \
                 tc.tile_pool(name="bps_s", bufs=sps_bufs, space="PSUM") as pss, \
                 tc.tile_pool(name="bps_m", bufs=2, space="PSUM") as psm:
                qtbs = {0: first_q}
                for qb in range(NQB):
                    qtb = qtbs.pop(qb)
                    # prefetch next q-block
                    if qb + 1 < NQB:
                        nxq = qp.tile([P, DT, QBLK], FP16, tag="qtb")
                        for d in range(DT):
                            nc.gpsimd.dma_start(
                                out=nxq[:, d],
                                in_=qt_d[d * P:(d + 1) * P,
                                         (qb + 1) * QBLK:(qb + 2) * QBLK])
                        qtbs[qb + 1] = nxq

                    probsT = ptp.tile([P, nkt, QBLK], FP16, tag="probsT")

                    def scores_softmax(qt):
                        sps = pss.tile([P, lkc], F32, tag="scps")
                        for (koff, kw) in kslices:
                            for i in range(DT):
                                nc.tensor.matmul(
                                    sps[:, koff:koff + kw],
                                    qtb[:, i, qt * P:(qt + 1) * P],
                                    z_slice(i, koff, kw),
                                    start=(i == 0), stop=(i == DT - 1))
                        s_t = smp.tile([P, lkc], F32, tag="s", bufs=3)
                        nc.vector.tensor_add(s_t, sps, biasrep_t)
                        mx = smp.tile([P, 1], F32, tag="mx")
                        nc.vector.reduce_max(mx, s_t, axis=AX.X)
                        negmx = smp.tile([P, 1], F32, tag="negmx")
                        nc.vector.tensor_scalar_mul(negmx, mx, -1.0)
                        p_t = smp.tile([P, lkc], FP16, tag="pp", bufs=3)
                        sume = smp.tile([P, 1], F32, tag="sume")
                        nc.scalar.activation(p_t, s_t, AF.Exp, bias=negmx, scale=1.0,
                                             accum_out=sume)
                        recip = smp.tile([P, 1], F32, tag="recip")
                        nc.vector.reciprocal(recip, sume)
                        nc.vector.tensor_scalar_mul(p_t, p_t, recip)
                        return p_t

                    def transposes(qt, p_t):
                        # XBAR DMA transpose: [128, lkc] -> [lkc(=P x nkt), 128]
                        # written straight into probsT; runs on the DMA
                        # engines, entirely off the PE
                        nc.sync.dma_start_transpose(
                            probsT[:, :, qt * P:(qt + 1) * P], p_t)

                    def av_half(half):
                        q0 = half * (QBLK // 2)
                        qw = QBLK // 2
                        for vt in range(DT):
                            ps = psm.tile([P, 512], F32, tag="mmps")
                            for kt in range(nkt):
                                nc.tensor.matmul(
                                    ps[:, 0:qw],
                                    vh_t[:, kt, vt * P:(vt + 1) * P],
                                    probsT[:, kt, q0:q0 + qw],
                                    start=(kt == 0), stop=(kt == nkt - 1))
                            ot = stp.tile([P, 512], FP16, tag="ot")
                            nc.vector.tensor_scalar_add(ot, ps[:, 0:qw],
                                                        bo2_t[:, vt:vt + 1])
                            eng = nc.gpsimd if vt % 2 == 0 else nc.scalar
                            eng.dma_start(
                                out=out_d[vt * P:(vt + 1) * P,
                                          qb * QBLK + q0:qb * QBLK + q0 + qw],
                                in_=ot)

                    # emission order keeps the PE fed across softmax chains
                    ps_ = {}
                    ps_[0] = scores_softmax(0)
                    ps_[1] = scores_softmax(1)
                    transposes(0, ps_[0])
                    ps_[2] = scores_softmax(2)
                    transposes(1, ps_[1])
                    ps_[3] = scores_softmax(3)
                    transposes(2, ps_[2])
                    ps_[4] = scores_softmax(4)
                    transposes(3, ps_[3])
                    ps_[5] = scores_softmax(5)
                    transposes(4, ps_[4])
                    ps_[6] = scores_softmax(6)
                    transposes(5, ps_[5])
                    ps_[7] = scores_softmax(7)
                    av_half(0)
                    transposes(6, ps_[6])
                    transposes(7, ps_[7])
                    av_half(1)
            bq1_cm.__exit__(None, None, None)
    nc.compile()
    return nc


_NC_CACHE = {}


def _get_nc(lkc=LKC_DEFAULT):
    if lkc not in _NC_CACHE:
        _NC_CACHE[lkc] = build_nc(lkc)
    return _NC_CACHE[lkc]


def _pad_up(n, m):
    return ((n + m - 1) // m) * m


def prepare(q, k, mask, Wq, bq, Wk, bk, Wv, bv, Wo, bo):
    """Returns (nc, in_maps) for run_bass_kernel_spmd."""
    q = np.asarray(q, np.float32)
    k = np.asarray(k, np.float32)
    mask = np.asarray(mask)
    Wq = np.asarray(Wq, np.float32)
    Wk = np.asarray(Wk, np.float32)
    Wv = np.asarray(Wv, np.float32)
    Wo = np.asarray(Wo, np.float32)
    bq_ = np.asarray(bq, np.float32)
    bk_ = np.asarray(bk, np.float32)
    bv_ = np.asarray(bv, np.float32)
    bo_ = np.asarray(bo, np.float32)

    nnz_max = int(mask.astype(bool).sum(axis=1).max())
    lkc = max(256, _pad_up(nnz_max, P))
    nc = _get_nc(lkc)

    # host-folded weights (weights-only GEMMs)
    at16 = (Wk @ Wq.T).astype(np.float16)                 # A.T,  A = Wq@Wk.T
    WvWo = Wv.astype(np.float64) @ Wo.astype(np.float64)
    ct16 = (Wk.astype(np.float64) @ WvWo).astype(np.float16)   # C = Wk@Wv@Wo
    bo2 = (bk_.astype(np.float64) @ WvWo
           + bv_.astype(np.float64) @ Wo.astype(np.float64) + bo_).astype(np.float32)
    w2 = Wk @ bq_                                          # c_k = kc . w2
    in_maps = []
    for b in range(B):
        idx = np.nonzero(mask[b])[0]
        kc = k[b][idx]
        ktc = np.zeros((D, lkc), np.float16)
        ktc[:, :len(idx)] = kc.T.astype(np.float16)
        brow = np.full(lkc, -10000.0, np.float32)
        brow[:len(idx)] = kc @ w2
        brep = np.broadcast_to(brow.astype(np.float16), (P, lkc)).copy()
        in_maps.append({
            "qt": np.ascontiguousarray(q[b].T.astype(np.float16)),
            "ktc": ktc,
            "at": at16, "ct": ct16,
            "bo2": np.ascontiguousarray(bo2.reshape(DT, P).T),
            "brep": brep,
        })
    return nc, in_maps


def kernel(q, k, mask, Wq, bq, Wk, bk, Wv, bv, Wo, bo):
    nc, in_maps = prepare(q, k, mask, Wq, bq, Wk, bk, Wv, bv, Wo, bo)
    res = run_bass_kernel_spmd(nc, in_maps, core_ids=list(range(B)))
    out = np.stack([np.ascontiguousarray(res.results[b]["out"].T) for b in range(B)])
    return out.astype(np.float32)
